# revision 9
# baseline (speedup 1.0000x reference)
"""GCN message-passing kernel for trn2, 8-core SPMD — v2.

Per core (dst-partitioned, 98 blocks of 128 dst nodes):
  Edges bucketed host-side into (src-chunk, dst-block) cells, each padded to
  NTC tiles of 128 tokens. Edge phase: big dma_gather batches (CPG cells per
  gather) from bf16 row tables -> per-tile one-hot [128x128] matmuls
  accumulated in a PSUM bank per cell -> vector-add into SBUF accumulators.
  No scatter-adds, no DRAM accumulator tables.
L1 output t1 stays in SBUF; z for own nodes built via batched rank-1 PE
matmuls; z AllGather; L2 same edge phase with elem=32; graph pooling via
one-hot matmuls + AllGather + full MLP on every core (as v1).
"""
import numpy as np
import ml_dtypes
import concourse.bass as bass
import concourse.bacc as bacc
import concourse.mybir as mybir
from concourse import tile, ap_utils
from concourse.bass import round_up_to_multiple, exact_div

F32 = mybir.dt.float32
BF16 = mybir.dt.bfloat16
I16 = mybir.dt.int16
I32 = mybir.dt.int32
AF = mybir.ActivationFunctionType
OP = mybir.AluOpType

N_NODES = 100000
N_GRAPHS = 2000
NN = 100096            # padded nodes = 782*128
NCOLS = 782
CORE_N = 12544         # nodes per core (98 blocks); core 7 has 12288 real
NBLK = 98              # dst blocks (128 nodes each) per core
CHUNK = 25088          # src chunk (int16-safe gather window)
N_CHUNKS = 4
TAB_ROWS = 100352      # 4*25088 = 784*128
CPG = 7                # cells per gather batch (98 = 14*7)
GPC = NBLK // CPG      # gather batches per chunk
GT = 8                 # tiles (128 idxs each) per dma_gather instruction
G_PAD = 2048
G_ASM = 2304


def raw_dma_gather(gp, out_ap, in_ap, idxs_ap, num_idxs, elem_size, queue_num=0,
                   single_packet=True):
    """dma_gather without the 256B elem_size restriction (non-transpose, HBM src)."""
    gp._assert_queue_num(queue_num)
    assert idxs_ap.dtype == I16
    assert in_ap.dtype == out_ap.dtype
    assert in_ap.ap[-1][1] == elem_size and out_ap.ap[-1][1] == elem_size
    assert out_ap.ap[0][1] * out_ap.ap[1][1] == round_up_to_multiple(num_idxs, 128)
    assert ap_utils.ap_is_contiguous(out_ap.ap[1:])
    assert ap_utils.ap_is_contiguous(idxs_ap.ap[1:])
    elem_step = in_ap.ap[0][0]
    stride_bytes = elem_step * mybir.dt.size(in_ap.dtype)
    stride_bytes_256 = exact_div(stride_bytes, 256)
    _in_ap = gp.lower_ap_dma(in_ap, for_custom_bir_dma=True)
    _idxs_ap = gp.lower_ap(idxs_ap)
    _out_ap = gp.lower_ap(out_ap)
    return gp.add_instruction(
        mybir.InstDMAGatherAnt(
            name=gp.bass.get_next_instruction_name(),
            ins=[*_in_ap, _idxs_ap, gp.lower_val_access(gp.to_reg(num_idxs))],
            outs=[_out_ap],
            transpose=False, num_idxs=num_idxs, elem_size=elem_size,
            stride_bytes_256=stride_bytes_256, gen_mode=0,
            single_packet=single_packet,
            queue_num=queue_num, sbuf_tokens_per_rank=0, sbuf_free_dim_per_rank=0,
            sbuf_free_dim_pad_per_rank=0, sbuf_byte_offset=0))


def build_nc(g_first, ntc, do_l1=True, do_l2=True, nq=4, gt=None, qrr=True,
             sp=True, upto='full', zf8=False,
             no_gather=False, no_oh=False, no_mm=False, no_acc=False,
             ps_bufs=4, tok_bufs=3, oh_bufs=2):
    gt = GT if gt is None else gt
    T = NBLK * ntc            # tiles per chunk
    G = CPG * ntc * 128       # tokens per gather batch
    ECOLS = T * 8             # es idx cols ([128, T*8]: 16-wrap, 8x replicated)

    nc = bacc.Bacc(None, target_bir_lowering=False, debug=False,
                   num_swdge_queues=nq)
    nc.num_devices = 8

    def Pm(name, shape, dt):
        return nc.declare_dram_parameter(name, shape, dt, isOutput=False)

    xg_p = Pm("xg_p", [128, NCOLS], F32)
    disg_p = Pm("disg_p", [128, NCOLS], F32)
    xo_p = Pm("xo_p", [128, NBLK], F32)
    diso_p = Pm("diso_p", [128, NBLK], F32)
    bid_p = Pm("bid_p", [128, NBLK], F32)
    counts = Pm("counts", [G_PAD], F32)
    w1 = Pm("w1", [64], F32)
    b1 = Pm("b1", [64], F32)
    W2 = Pm("W2", [64, 32], F32)
    b2 = Pm("b2", [32], F32)
    Wp1 = Pm("Wp1", [32, 128], F32)
    bp1 = Pm("bp1", [128], F32)
    Wp2 = Pm("Wp2", [128, 3], F32)
    bp2 = Pm("bp2", [3], F32)
    esrc = Pm("esrc", [N_CHUNKS, 128, ECOLS], I16)
    dstw = Pm("dstw", [N_CHUNKS, 128, T], BF16)
    out = nc.declare_dram_parameter("out", [N_GRAPHS, 3], F32, isOutput=True)

    y_tab = nc.dram_tensor("y_tab", [TAB_ROWS, 128], BF16)
    ZDT = mybir.dt.float8e4 if zf8 else BF16
    ZW = 256 if zf8 else 128
    z_own = nc.dram_tensor("z_own", [CORE_N, ZW], ZDT)
    z_tab = nc.dram_tensor("z_tab", [TAB_ROWS, ZW], ZDT, addr_space="Shared")
    s_dram = nc.dram_tensor("s_dram", [CORE_N], F32)
    cc_in = nc.dram_tensor("cc_in", [32, 512], F32)
    cc_out = nc.dram_tensor("cc_out", [8 * 32, 512], F32, addr_space="Shared")

    with tile.TileContext(nc) as tc:
        with tc.tile_pool(name="const", bufs=1) as cp, \
             tc.tile_pool(name="work", bufs=3) as wp:
            ap_pool = tc.tile_pool(name="phaseA", bufs=1)
            ap = ap_pool.__enter__()

            # ---------- Phase A: constants + y table ----------
            io512i = ap.tile([128, 512], I32)
            nc.gpsimd.iota(io512i[:], pattern=[[1, 512]], base=0, channel_multiplier=0)
            io512 = cp.tile([128, 512], F32)
            nc.vector.tensor_copy(io512[:], io512i[:])
            iopi = ap.tile([128, 1], I32)
            nc.gpsimd.iota(iopi[:], pattern=[[0, 1]], base=0, channel_multiplier=1)
            iop = ap.tile([128, 1], F32)
            nc.vector.tensor_copy(iop[:], iopi[:])
            io128i = ap.tile([128, 128], I32)
            nc.gpsimd.iota(io128i[:], pattern=[[1, 128]], base=0, channel_multiplier=0)
            io128 = ap.tile([128, 128], F32)
            nc.vector.tensor_copy(io128[:], io128i[:])
            io128b = cp.tile([128, 128], BF16)
            nc.vector.tensor_copy(io128b[:], io128i[:])
            ident = cp.tile([128, 128], F32)
            nc.vector.tensor_scalar(out=ident[:], in0=io128[:], scalar1=iop[:],
                                    scalar2=None, op0=OP.is_equal)
            ones1 = cp.tile([1, 128], F32)
            nc.vector.memset(ones1[:], 1.0)

            w1r = cp.tile([1, 64], F32)
            nc.sync.dma_start(out=w1r[:], in_=w1[:].unsqueeze(0))
            b2r = ap.tile([1, 32], F32)
            nc.sync.dma_start(out=b2r[:], in_=b2[:].unsqueeze(0))
            b2b = cp.tile([128, 32], F32)
            with tc.tile_pool(name="psA", bufs=1, space="PSUM") as psA:
                bc = psA.tile([128, 32], F32)
                nc.tensor.matmul(bc[:], ones1[:], b2r[:], start=True, stop=True)
                nc.scalar.activation(b2b[:], bc[:], AF.Copy)

            W2sb = ap.tile([64, 32], F32)
            nc.sync.dma_start(out=W2sb[:], in_=W2[:, :])
            W2b = cp.tile([64, 32], BF16)
            nc.vector.tensor_copy(W2b[:], W2sb[:])
            Wp1sb = cp.tile([32, 128], F32)
            nc.sync.dma_start(out=Wp1sb[:], in_=Wp1[:, :])
            Wp2sb = cp.tile([128, 3], F32)
            nc.sync.dma_start(out=Wp2sb[:], in_=Wp2[:, :])
            bp1c = cp.tile([128, 1], F32)
            nc.sync.dma_start(out=bp1c[:], in_=bp1[:].unsqueeze(1))
            bp2c = cp.tile([3, 1], F32)
            nc.sync.dma_start(out=bp2c[:], in_=bp2[:].unsqueeze(1))
            b1c = cp.tile([64, 1], F32)
            nc.sync.dma_start(out=b1c[:], in_=b1[:].unsqueeze(1))

            # global node vectors (p-major tiles straight from HBM)
            xg = ap.tile([128, NCOLS], F32)
            nc.sync.dma_start(out=xg[:], in_=xg_p[:, :])
            disg = ap.tile([128, NCOLS], F32)
            nc.sync.dma_start(out=disg[:], in_=disg_p[:, :])
            yg = ap.tile([128, NCOLS], F32)
            nc.vector.tensor_tensor(out=yg[:], in0=disg[:], in1=xg[:], op=OP.mult)
            y2 = ap.tile([128, NCOLS * 2], BF16)
            nc.vector.memset(y2[:], 0.0)
            y23 = y2[:].rearrange("p (f t) -> p f t", t=2)
            nc.vector.tensor_copy(y23[:, :, 0:1], yg[:].unsqueeze(2))
            FPC = CHUNK // 128   # 196 f-columns per chunk
            ytv = y_tab[:, :].rearrange("(f p) c -> p f c", p=128)
            for c in range(N_CHUNKS):
                f0, f1 = FPC * c, min(FPC * (c + 1), NCOLS)
                nc.sync.dma_start(out=ytv[:, f0:f1, 0:2], in_=y23[:, f0:f1, :])
            ypad = ap.tile([128, 4], BF16)
            nc.vector.memset(ypad[:], 0.0)
            nc.sync.dma_start(
                out=ytv[:, NCOLS:784, 0:2],
                in_=ypad[:].rearrange("p (f t) -> p f t", t=2))

            xo = cp.tile([128, NBLK], F32)
            nc.sync.dma_start(out=xo[:], in_=xo_p[:, :])
            diso = cp.tile([128, NBLK], F32)
            nc.sync.dma_start(out=diso[:], in_=diso_p[:, :])
            bidc = cp.tile([128, NBLK], F32)
            nc.sync.dma_start(out=bidc[:], in_=bid_p[:, :])

            # SBUF accumulators for the two aggregations
            t1_sb = cp.tile([128, NBLK * 2], F32)
            nc.vector.memset(t1_sb[:], 0.0)
            t2_sb = cp.tile([128, NBLK * 32], F32)
            nc.vector.memset(t2_sb[:], 0.0)

            ap_pool.__exit__(None, None, None)

            # ---------- edge phase ----------
            def edge_phase(tab, acc_sb, elem, tdt=BF16):
                with tc.tile_pool(name="psE", bufs=ps_bufs, space="PSUM") as psE, \
                     tc.tile_pool(name="chunkdat", bufs=3) as kp, \
                     tc.tile_pool(name="tok", bufs=tok_bufs) as tp, \
                     tc.tile_pool(name="ohp", bufs=oh_bufs) as op_:
                    acc3 = acc_sb[:].rearrange("p (j e) -> p j e", e=elem)
                    for c in range(N_CHUNKS):
                        es = kp.tile([128, ECOLS], I16, tag="es")
                        nc.sync.dma_start(out=es[:], in_=esrc[c])
                        dw = kp.tile([128, T], BF16, tag="dw")
                        nc.sync.dma_start(out=dw[:], in_=dstw[c])
                        tab_c = tab[c * CHUNK:(c + 1) * CHUNK, 0:elem]
                        for g in range(GPC):
                            nt = CPG * ntc
                            tok = tp.tile([128, nt * elem], tdt, tag="tok")
                            tok3 = tok[:].rearrange("p (t e) -> p t e", e=elem)
                            if not no_gather:
                                for qi, q0 in enumerate(range(0, nt, gt)):
                                    q1 = min(q0 + gt, nt)
                                    qn = (qi % nq) if qrr else 0
                                    raw_dma_gather(
                                        nc.gpsimd, tok3[:, q0:q1, :], tab_c,
                                        es[:, g * (G // 16) + q0 * 8:
                                           g * (G // 16) + q1 * 8],
                                        (q1 - q0) * 128, elem, queue_num=qn,
                                        single_packet=sp)
                            if not no_oh:
                                oh = op_.tile([128, nt * 128], tdt, tag="oh")
                                oh3 = oh[:].rearrange("p (t w) -> p t w", w=128)
                                nc.vector.tensor_tensor(
                                    out=oh3,
                                    in0=dw[:, g * nt:(g + 1) * nt].unsqueeze(2)
                                        .broadcast_to([128, nt, 128]),
                                    in1=io128b[:].unsqueeze(1)
                                        .broadcast_to([128, nt, 128]),
                                    op=OP.is_equal)
                            else:
                                oh3 = io128b[:].unsqueeze(1).broadcast_to(
                                    [128, nt, 128])
                            for cell in range(CPG):
                                j = g * CPG + cell
                                ps = psE.tile([128, elem], F32, tag="cell")
                                ps3 = ps[:].unsqueeze(1)
                                if not no_mm:
                                    for t in range(ntc):
                                        tt = cell * ntc + t
                                        nc.tensor.matmul(
                                            ps3, oh3[:, tt:tt + 1, :],
                                            tok3[:, tt:tt + 1, :],
                                            start=(t == 0), stop=(t == ntc - 1))
                                if not (no_acc or no_mm):
                                    nc.vector.tensor_tensor(
                                        out=acc3[:, j:j + 1, :],
                                        in0=acc3[:, j:j + 1, :],
                                        in1=ps3, op=OP.add)

            # ---------- Phase B: L1 ----------
            if do_l1 and upto in ('B', 'C', 'D', 'full'):
                edge_phase(y_tab, t1_sb, 2)

            # ---------- Phase C: z for own nodes ----------
            do_c = upto in ('C', 'D', 'full')
            cp_pool = tc.tile_pool(name="phaseC", bufs=1)
            ep = cp_pool.__enter__()
            zsb = cp.tile([128, NBLK * 32], F32)
            zs3 = zsb[:].rearrange("p (f e) -> p f e", e=32)
            if not do_c:
                nc.vector.memset(zsb[:], 0.0)
            if do_c:
                t13 = t1_sb[:].rearrange("p (j e) -> p j e", e=2)
                d2 = ep.tile([128, NBLK], F32)
                nc.vector.tensor_tensor(out=d2[:], in0=diso[:], in1=diso[:], op=OP.mult)
                nc.vector.tensor_tensor(out=d2[:], in0=d2[:], in1=xo[:], op=OP.mult)
                s = ep.tile([128, NBLK], F32)
                nc.vector.tensor_tensor(out=s[:].unsqueeze(2), in0=t13[:, :, 0:1],
                                        in1=diso[:].unsqueeze(2), op=OP.mult)
                nc.vector.tensor_tensor(out=s[:], in0=s[:], in1=d2[:], op=OP.add)

                # s -> single-partition row (node-order) via SBUF->SBUF DMA
                s_row = ep.tile([1, CORE_N], F32)
                nc.sync.dma_start(out=s_dram[:].rearrange("(f p) -> p f", p=128),
                                  in_=s[:])
                nc.sync.dma_start(out=s_row[:], in_=s_dram[:].unsqueeze(0))

                # h1T[k, n] = relu(w1[k]*s[n] + b1[k]) built 512 nodes at a time
                h1rT = ep.tile([64, CORE_N], BF16)
                with tc.tile_pool(name="psH", bufs=3, space="PSUM") as psH:
                    for j0 in range(0, CORE_N, 512):
                        n = min(512, CORE_N - j0)
                        h1_ps = psH.tile([64, 512], F32, tag="h1")
                        nc.tensor.matmul(h1_ps[:, 0:n], w1r[:],
                                         s_row[:, j0:j0 + n],
                                         start=True, stop=True)
                        nc.scalar.activation(h1rT[:, j0:j0 + n],
                                             h1_ps[:, 0:n], AF.Relu, bias=b1c[:],
                                             scale=1.0)

                # z[n,:] = diso[n] * (h1r @ W2)[n,:]
                z2 = ep.tile([128, NBLK * 32], ZDT)
                z23 = z2[:].rearrange("p (f e) -> p f e", e=32)
                with tc.tile_pool(name="psC", bufs=3, space="PSUM") as psC:
                    for f in range(NBLK):
                        z_ps = psC.tile([128, 32], F32, tag="zps")
                        nc.tensor.matmul(z_ps[:], h1rT[:, 128 * f:128 * (f + 1)],
                                         W2b[:], start=True, stop=True)
                        nc.vector.tensor_scalar(out=zs3[:, f:f + 1, :],
                                                in0=z_ps[:].unsqueeze(1),
                                                scalar1=diso[:, f:f + 1], scalar2=None,
                                                op0=OP.mult)
                        nc.vector.tensor_copy(z23[:, f:f + 1, :], zs3[:, f:f + 1, :])
                nc.sync.dma_start(
                    out=z_own[:, :].rearrange("(f p) c -> p f c", p=128)[:, :, 0:32],
                    in_=z23)
                nc.gpsimd.collective_compute(
                    "AllGather", OP.bypass, replica_groups=[list(range(8))],
                    ins=[z_own[:, :].opt()], outs=[z_tab[:, :].opt()])
            cp_pool.__exit__(None, None, None)

            # ---------- Phase D: L2 ----------
            if do_l2 and upto in ('D', 'full'):
                edge_phase(z_tab, t2_sb, 32, tdt=ZDT)

            # ---------- Phase E: h2, pooling, MLP ----------
            ep_pool = tc.tile_pool(name="phaseE", bufs=1)
            ep = ep_pool.__enter__()
            t23 = t2_sb[:].rearrange("p (f e) -> p f e", e=32)
            hf = ep.tile([128, NBLK * 32], F32)
            hf3 = hf[:].rearrange("p (f e) -> p f e", e=32)
            nc.vector.tensor_tensor(out=hf3, in0=t23, in1=zs3, op=OP.add)
            nc.vector.tensor_tensor(out=hf3, in0=hf3,
                                    in1=diso[:].unsqueeze(2)
                                    .broadcast_to([128, NBLK, 32]), op=OP.mult)
            nc.vector.tensor_tensor(out=hf3, in0=hf3,
                                    in1=b2b[:].unsqueeze(1)
                                    .broadcast_to([128, NBLK, 32]), op=OP.add)
            nc.vector.tensor_scalar(out=hf[:], in0=hf[:], scalar1=0.0, scalar2=None,
                                    op0=OP.max)

            with tc.tile_pool(name="psP", bufs=1, space="PSUM") as psP, \
                 tc.tile_pool(name="psM", bufs=2, space="PSUM") as psM:
                pool_ps = psP.tile([32, 512], F32)
                for f in range(NBLK):
                    oh = wp.tile([128, 512], F32, tag="poh")
                    nc.vector.tensor_scalar(out=oh[:], in0=io512[:],
                                            scalar1=bidc[:, f:f + 1], scalar2=None,
                                            op0=OP.is_equal)
                    nc.tensor.matmul(pool_ps[:], hf3[:, f, :], oh[:],
                                     start=(f == 0), stop=(f == NBLK - 1))
                poolsb = ep.tile([32, 512], F32)
                nc.scalar.activation(poolsb[:], pool_ps[:], AF.Copy)
                nc.sync.dma_start(out=cc_in[:, :], in_=poolsb[:])
                nc.gpsimd.collective_compute(
                    "AllGather", OP.bypass, replica_groups=[list(range(8))],
                    ins=[cc_in[:, :].opt()], outs=[cc_out[:, :].opt()])

                pooled = ep.tile([32, G_ASM], F32)
                nc.vector.memset(pooled[:], 0.0)
                for c2 in range(8):
                    slab = wp.tile([32, 512], F32, tag="slab")
                    nc.sync.dma_start(out=slab[:],
                                      in_=cc_out[32 * c2:32 * (c2 + 1), :])
                    g0 = g_first[c2]
                    nc.vector.tensor_tensor(out=pooled[:, g0:g0 + 512],
                                            in0=pooled[:, g0:g0 + 512],
                                            in1=slab[:], op=OP.add)

                cnt = ep.tile([1, G_PAD], F32)
                nc.sync.dma_start(out=cnt[:], in_=counts[:].unsqueeze(0))
                nc.vector.tensor_scalar(out=cnt[:], in0=cnt[:], scalar1=1.0,
                                        scalar2=None, op0=OP.max)
                crec = ep.tile([1, G_PAD], F32)
                nc.vector.reciprocal(crec[:], cnt[:])
                crep = ep.tile([32, G_PAD], F32)
                for j in range(4):
                    cr_ps = psM.tile([32, 512], F32, tag="mm")
                    nc.tensor.matmul(cr_ps[:], ones1[:, 0:32],
                                     crec[:, 512 * j:512 * (j + 1)],
                                     start=True, stop=True)
                    nc.scalar.activation(crep[:, 512 * j:512 * (j + 1)], cr_ps[:],
                                         AF.Copy)
                pm = ep.tile([32, G_PAD], F32)
                nc.vector.tensor_tensor(out=pm[:], in0=pooled[:, 0:G_PAD],
                                        in1=crep[:], op=OP.mult)

                m1 = ep.tile([128, G_PAD], F32)
                for j in range(4):
                    m1_ps = psM.tile([128, 512], F32, tag="mm")
                    nc.tensor.matmul(m1_ps[:], Wp1sb[:],
                                     pm[:, 512 * j:512 * (j + 1)],
                                     start=True, stop=True)
                    nc.scalar.activation(m1[:, 512 * j:512 * (j + 1)], m1_ps[:],
                                         AF.Relu, bias=bp1c[:], scale=1.0)
                osb = ep.tile([3, G_PAD], F32)
                for j in range(4):
                    m2_ps = psM.tile([128, 512], F32, tag="mm")
                    nc.tensor.matmul(m2_ps[0:3, :], Wp2sb[:],
                                     m1[:, 512 * j:512 * (j + 1)],
                                     start=True, stop=True)
                    nc.vector.tensor_scalar(out=osb[:, 512 * j:512 * (j + 1)],
                                            in0=m2_ps[0:3, :], scalar1=bp2c[:],
                                            scalar2=None, op0=OP.add)
            nc.sync.dma_start(out=out[:, :].rearrange("g e -> e g"),
                              in_=osb[:, 0:N_GRAPHS])
            ep_pool.__exit__(None, None, None)
    nc.compile()
    return nc


# ---------------- host-side preprocessing ----------------

def prep_host(edge_index, batch):
    """Bucket edges into (chunk, block) cells; build idx/offset streams."""
    src = np.asarray(edge_index[0], np.int64)
    dst = np.asarray(edge_index[1], np.int64)
    batch = np.asarray(batch, np.int64)
    indeg = np.bincount(dst, minlength=NN).astype(np.int32)
    NCELL = N_CHUNKS * NBLK

    core_of = dst // CORE_N
    per_core_raw = []
    ntc = 1
    for c in range(8):
        m = core_of == c
        s_c = src[m]
        dl = dst[m] - c * CORE_N
        chunk = s_c // CHUNK
        block = dl >> 7
        off = dl & 127
        cell = chunk * NBLK + block
        cnts = np.bincount(cell, minlength=NCELL)
        ntc = max(ntc, int(-(-cnts.max() // 128)))
        per_core_raw.append((s_c, chunk, off, cell, cnts))

    cap = ntc * 128
    g_first = []
    per_core = []
    for c in range(8):
        s_c, chunk, off, cell, cnts = per_core_raw[c]
        # sort by (cell, src) so each cell's gather sweeps the table in
        # ascending row order -> HBM row-buffer locality
        order = np.lexsort((s_c, cell))
        cs = cell[order]
        base = np.zeros(NCELL + 1, np.int64)
        np.cumsum(cnts, out=base[1:])
        rank = np.arange(len(cs)) - base[cs]
        pos = cs * cap + rank
        es_stream = np.zeros(NCELL * cap, np.int16)
        es_stream[pos] = (s_c[order] - chunk[order] * CHUNK).astype(np.int16)
        dw_stream = np.full(NCELL * cap, -1.0, ml_dtypes.bfloat16)
        dw_stream[pos] = off[order].astype(ml_dtypes.bfloat16)

        esw = np.tile(es_stream.reshape(N_CHUNKS, -1, 16).transpose(0, 2, 1),
                      (1, 8, 1)).copy()
        dww = dw_stream.reshape(N_CHUNKS, -1, 128).transpose(0, 2, 1).copy()

        nb_real = min(CORE_N, N_NODES - c * CORE_N)
        bid_own = np.full(CORE_N, -1.0, np.float32)
        gf = int(batch[c * CORE_N])
        bid_own[:nb_real] = (batch[c * CORE_N:c * CORE_N + nb_real] - gf).astype(
            np.float32)
        assert bid_own.max() < 512
        g_first.append(gf)
        per_core.append(dict(esrc=esw, dstw=dww,
                             bid_p=np.ascontiguousarray(
                                 bid_own.reshape(NBLK, 128).T),
                             nb_real=nb_real))

    counts = np.bincount(batch, minlength=G_PAD).astype(np.float32)[:G_PAD]
    dis = ((indeg.astype(np.float64) + 1.0) ** -0.5).astype(np.float32)
    return per_core, dict(indeg=indeg, dis=dis, counts=counts, g_first=g_first,
                          ntc=ntc)


def make_inmaps(inputs, per_core, uniform):
    """Build per-core in_maps from full problem inputs + prep results."""
    x = np.asarray(inputs["x"], np.float32).reshape(-1)
    x_pad = np.zeros(NN, np.float32)
    x_pad[:N_NODES] = x
    xg_p = np.ascontiguousarray(x_pad.reshape(NCOLS, 128).T)
    disg_p = np.ascontiguousarray(uniform["dis"].reshape(NCOLS, 128).T)
    common = dict(
        xg_p=xg_p, disg_p=disg_p,
        counts=uniform["counts"],
        w1=np.asarray(inputs["W1"], np.float32).reshape(64),
        b1=np.asarray(inputs["b1"], np.float32),
        W2=np.asarray(inputs["W2"], np.float32),
        b2=np.asarray(inputs["b2"], np.float32),
        Wp1=np.asarray(inputs["Wp1"], np.float32),
        bp1=np.asarray(inputs["bp1"], np.float32),
        Wp2=np.asarray(inputs["Wp2"], np.float32),
        bp2=np.asarray(inputs["bp2"], np.float32),
    )
    dis_pad = np.ones(NN, np.float32)
    dis_pad[:] = uniform["dis"][:NN]
    in_maps = []
    for c in range(8):
        pc = per_core[c]
        xo = np.zeros(CORE_N, np.float32)
        nb = pc["nb_real"]
        xo[:nb] = x_pad[c * CORE_N:c * CORE_N + nb]
        dso = np.ones(CORE_N, np.float32)
        dso[:nb] = dis_pad[c * CORE_N:c * CORE_N + nb]
        in_maps.append(dict(
            common,
            xo_p=np.ascontiguousarray(xo.reshape(NBLK, 128).T),
            diso_p=np.ascontiguousarray(dso.reshape(NBLK, 128).T),
            bid_p=pc["bid_p"], esrc=pc["esrc"], dstw=pc["dstw"]))
    return in_maps


# ---------------- harness entry point ----------------

_CACHE = {}


def kernel(**inputs):
    """Full-input GCN forward on 8 trn2 NeuronCores; returns [2000, 3] f32."""
    from concourse.bass_utils import run_bass_kernel_spmd
    inputs = {k: np.asarray(v) for k, v in inputs.items()}
    per_core, uniform = prep_host(inputs["edge_index"], inputs["batch"])
    key = (tuple(uniform["g_first"]), uniform["ntc"])
    if key not in _CACHE:
        _CACHE[key] = build_nc(uniform["g_first"], uniform["ntc"])
    nc = _CACHE[key]
    in_maps = make_inmaps(inputs, per_core, uniform)
    res = run_bass_kernel_spmd(nc, in_maps, core_ids=list(range(8)))
    return np.ascontiguousarray(res.results[0]["out"].astype(np.float32))



# revision 18
# speedup vs baseline: 1.0205x; 1.0205x over previous
"""GCN message-passing kernel for trn2, 8-core SPMD — v3 (packed tables).

Per core (dst-partitioned, 98 blocks of 128 dst nodes):
  Edges bucketed host-side into (src%4 class, dst-block) cells, each padded
  to NTC tiles of 128 tokens. Tables pack 4 nodes per 256B row (row =
  src//4, class = src%4); gathers use per-class column windows so row
  indices fit int16 with no chunking, and the z AllGather moves only the
  packed 802KB/core instead of a 256B-strided table (4x less traffic).
  Edge phase: dma_gather batches -> per-tile one-hot [128x128] matmuls
  accumulated in a PSUM bank per cell -> vector-add into SBUF accumulators.
L1 output t1 stays in SBUF; z for own nodes built via batched rank-1 PE
matmuls; packed z AllGather; L2 same edge phase with elem=32; graph pooling
via one-hot matmuls + AllGather + full MLP on every core.
"""
import numpy as np
import ml_dtypes
import concourse.bass as bass
import concourse.bacc as bacc
import concourse.mybir as mybir
from concourse import tile, ap_utils
from concourse.bass import round_up_to_multiple, exact_div

F32 = mybir.dt.float32
BF16 = mybir.dt.bfloat16
I16 = mybir.dt.int16
I32 = mybir.dt.int32
AF = mybir.ActivationFunctionType
OP = mybir.AluOpType

N_NODES = 100000
N_GRAPHS = 2000
NN = 100096            # padded nodes = 782*128
NCOLS = 782
CORE_N = 12544         # nodes per core (98 blocks); core 7 has 12192 real
NBLK = 98              # dst blocks (128 nodes each) per core
N_CLASSES = 4          # src % 4 -> column window within a 256B table row
ROWS = 25088           # packed table rows (src//4 < 25024)
CPG = 7                # cells per gather batch (98 = 14*7)
GPC = NBLK // CPG      # gather batches per class
GT = 8                 # tiles (128 idxs each) per dma_gather instruction
G_PAD = 2048
G_ASM = 2304


def raw_dma_gather(gp, out_ap, in_ap, idxs_ap, num_idxs, elem_size, queue_num=0,
                   single_packet=True):
    """dma_gather without the 256B elem_size restriction (non-transpose, HBM src)."""
    gp._assert_queue_num(queue_num)
    assert idxs_ap.dtype == I16
    assert in_ap.dtype == out_ap.dtype
    assert in_ap.ap[-1][1] == elem_size and out_ap.ap[-1][1] == elem_size
    assert out_ap.ap[0][1] * out_ap.ap[1][1] == round_up_to_multiple(num_idxs, 128)
    assert ap_utils.ap_is_contiguous(out_ap.ap[1:])
    assert ap_utils.ap_is_contiguous(idxs_ap.ap[1:])
    elem_step = in_ap.ap[0][0]
    stride_bytes = elem_step * mybir.dt.size(in_ap.dtype)
    stride_bytes_256 = exact_div(stride_bytes, 256)
    _in_ap = gp.lower_ap_dma(in_ap, for_custom_bir_dma=True)
    _idxs_ap = gp.lower_ap(idxs_ap)
    _out_ap = gp.lower_ap(out_ap)
    return gp.add_instruction(
        mybir.InstDMAGatherAnt(
            name=gp.bass.get_next_instruction_name(),
            ins=[*_in_ap, _idxs_ap, gp.lower_val_access(gp.to_reg(num_idxs))],
            outs=[_out_ap],
            transpose=False, num_idxs=num_idxs, elem_size=elem_size,
            stride_bytes_256=stride_bytes_256, gen_mode=0,
            single_packet=single_packet,
            queue_num=queue_num, sbuf_tokens_per_rank=0, sbuf_free_dim_per_rank=0,
            sbuf_free_dim_pad_per_rank=0, sbuf_byte_offset=0))


def build_nc(g_first, ntc, do_l1=True, do_l2=True, nq=4, gt=None, qrr=True,
             sp=True, upto='full', zf8=False,
             no_gather=False, no_oh=False, no_mm=False, no_acc=False,
             ps_bufs=4, tok_bufs=3, oh_bufs=2):
    gt = GT if gt is None else gt
    T = NBLK * ntc            # tiles per class
    G = CPG * ntc * 128       # tokens per gather batch
    ECOLS = T * 8             # es idx cols ([128, T*8]: 16-wrap, 8x replicated)

    nc = bacc.Bacc(None, target_bir_lowering=False, debug=False,
                   num_swdge_queues=nq)
    nc.num_devices = 8

    def Pm(name, shape, dt):
        return nc.declare_dram_parameter(name, shape, dt, isOutput=False)

    xg_p = Pm("xg_p", [128, NCOLS + 2], F32)   # packed layout [P', 4F'+m]
    disg_p = Pm("disg_p", [128, NCOLS + 2], F32)
    xo_p = Pm("xo_p", [128, NBLK], F32)
    diso_p = Pm("diso_p", [128, NBLK], F32)
    bid_p = Pm("bid_p", [128, NBLK], F32)
    counts = Pm("counts", [G_PAD], F32)
    w1 = Pm("w1", [64], F32)
    b1 = Pm("b1", [64], F32)
    W2 = Pm("W2", [64, 32], F32)
    b2 = Pm("b2", [32], F32)
    Wp1 = Pm("Wp1", [32, 128], F32)
    bp1 = Pm("bp1", [128], F32)
    Wp2 = Pm("Wp2", [128, 3], F32)
    bp2 = Pm("bp2", [3], F32)
    esrc = Pm("esrc", [N_CLASSES, 128, ECOLS], I16)
    dstw = Pm("dstw", [N_CLASSES, 128, T], BF16)
    out = nc.declare_dram_parameter("out", [N_GRAPHS, 3], F32, isOutput=True)

    y_tab = nc.dram_tensor("y_tab", [ROWS, 128], BF16)
    z_own = nc.dram_tensor("z_own", [CORE_N, 32], BF16)
    z_tab = nc.dram_tensor("z_tab", [ROWS, 128], BF16, addr_space="Shared")
    s_dram = nc.dram_tensor("s_dram", [CORE_N], F32)
    cc_in = nc.dram_tensor("cc_in", [32, 512], F32)
    cc_out = nc.dram_tensor("cc_out", [8 * 32, 512], F32, addr_space="Shared")

    FP = NN // 512 + 1        # 196 row-groups of 128 rows (4 nodes each)

    with tile.TileContext(nc) as tc:
        with tc.tile_pool(name="const", bufs=1) as cp, \
             tc.tile_pool(name="work", bufs=3) as wp:
            ap_pool = tc.tile_pool(name="phaseA", bufs=1)
            ap = ap_pool.__enter__()

            # ---------- Phase A: constants + y table ----------
            io512i = ap.tile([128, 512], I32)
            nc.gpsimd.iota(io512i[:], pattern=[[1, 512]], base=0, channel_multiplier=0)
            io512 = cp.tile([128, 512], mybir.dt.float16)
            nc.vector.tensor_copy(io512[:], io512i[:])
            iopi = ap.tile([128, 1], I32)
            nc.gpsimd.iota(iopi[:], pattern=[[0, 1]], base=0, channel_multiplier=1)
            iop = ap.tile([128, 1], F32)
            nc.vector.tensor_copy(iop[:], iopi[:])
            io128i = ap.tile([128, 128], I32)
            nc.gpsimd.iota(io128i[:], pattern=[[1, 128]], base=0, channel_multiplier=0)
            io128 = ap.tile([128, 128], F32)
            nc.vector.tensor_copy(io128[:], io128i[:])
            io128b = cp.tile([128, 128], BF16)
            nc.vector.tensor_copy(io128b[:], io128i[:])
            ident = cp.tile([128, 128], F32)
            nc.vector.tensor_scalar(out=ident[:], in0=io128[:], scalar1=iop[:],
                                    scalar2=None, op0=OP.is_equal)
            ones1 = cp.tile([1, 128], F32)
            nc.vector.memset(ones1[:], 1.0)

            w1r = cp.tile([1, 64], F32)
            nc.sync.dma_start(out=w1r[:], in_=w1[:].unsqueeze(0))
            b2r = ap.tile([1, 32], F32)
            nc.sync.dma_start(out=b2r[:], in_=b2[:].unsqueeze(0))
            b2b = cp.tile([128, 32], F32)
            with tc.tile_pool(name="psA", bufs=1, space="PSUM") as psA:
                bc = psA.tile([128, 32], F32)
                nc.tensor.matmul(bc[:], ones1[:], b2r[:], start=True, stop=True)
                nc.scalar.activation(b2b[:], bc[:], AF.Copy)

            W2sb = ap.tile([64, 32], F32)
            nc.sync.dma_start(out=W2sb[:], in_=W2[:, :])
            W2b = cp.tile([64, 32], BF16)
            nc.vector.tensor_copy(W2b[:], W2sb[:])
            Wp1sb = cp.tile([32, 128], F32)
            nc.sync.dma_start(out=Wp1sb[:], in_=Wp1[:, :])
            Wp2sb = cp.tile([128, 3], F32)
            nc.sync.dma_start(out=Wp2sb[:], in_=Wp2[:, :])
            bp1c = cp.tile([128, 1], F32)
            nc.sync.dma_start(out=bp1c[:], in_=bp1[:].unsqueeze(1))
            bp2c = cp.tile([3, 1], F32)
            nc.sync.dma_start(out=bp2c[:], in_=bp2[:].unsqueeze(1))
            b1c = cp.tile([64, 1], F32)
            nc.sync.dma_start(out=b1c[:], in_=b1[:].unsqueeze(1))

            # global node vectors, packed layout: col 4F+m, row F*128+P
            xg = ap.tile([128, NCOLS + 2], F32)
            nc.sync.dma_start(out=xg[:], in_=xg_p[:, :])
            disg = ap.tile([128, NCOLS + 2], F32)
            nc.sync.dma_start(out=disg[:], in_=disg_p[:, :])
            yg = ap.tile([128, NCOLS + 2], F32)
            nc.vector.tensor_tensor(out=yg[:], in0=disg[:], in1=xg[:], op=OP.mult)
            ygb = ap.tile([128, FP * 4], BF16)
            nc.vector.memset(ygb[:], 0.0)
            nc.vector.tensor_copy(ygb[:, 0:NCOLS + 2], yg[:])
            # y_tab[F*128+P, m] = ygb[P, 4F+m]
            nc.sync.dma_start(
                out=y_tab[0:FP * 128, 0:4].rearrange("(F p) m -> p F m", p=128),
                in_=ygb[:].rearrange("p (F m) -> p F m", m=4))

            xo = cp.tile([128, NBLK], F32)
            nc.sync.dma_start(out=xo[:], in_=xo_p[:, :])
            diso = cp.tile([128, NBLK], F32)
            nc.sync.dma_start(out=diso[:], in_=diso_p[:, :])
            bidc = cp.tile([128, NBLK], F32)
            nc.sync.dma_start(out=bidc[:], in_=bid_p[:, :])

            # SBUF accumulators for the two aggregations
            t1_sb = cp.tile([128, NBLK], F32)
            nc.vector.memset(t1_sb[:], 0.0)
            t2_sb = cp.tile([128, NBLK * 32], F32)
            nc.vector.memset(t2_sb[:], 0.0)

            ap_pool.__exit__(None, None, None)

            # ---------- edge phase ----------
            def edge_phase(tab, acc_sb, elem, tdt=BF16):
                with tc.tile_pool(name="psE", bufs=ps_bufs, space="PSUM") as psE, \
                     tc.tile_pool(name="chunkdat", bufs=3) as kp, \
                     tc.tile_pool(name="tok", bufs=tok_bufs) as tp, \
                     tc.tile_pool(name="ohp", bufs=oh_bufs) as op_:
                    acc3 = acc_sb[:].rearrange("p (j e) -> p j e", e=elem)
                    for c in range(N_CLASSES):
                        es = kp.tile([128, ECOLS], I16, tag="es")
                        nc.sync.dma_start(out=es[:], in_=esrc[c])
                        dw = kp.tile([128, T], BF16, tag="dw")
                        nc.sync.dma_start(out=dw[:], in_=dstw[c])
                        tab_c = tab[0:ROWS, elem * c:elem * (c + 1)]
                        for g in range(GPC):
                            nt = CPG * ntc
                            tok = tp.tile([128, nt * elem], tdt, tag="tok")
                            tok3 = tok[:].rearrange("p (t e) -> p t e", e=elem)
                            if not no_gather:
                                for qi, q0 in enumerate(range(0, nt, gt)):
                                    q1 = min(q0 + gt, nt)
                                    qn = (qi % nq) if qrr else 0
                                    raw_dma_gather(
                                        nc.gpsimd, tok3[:, q0:q1, :], tab_c,
                                        es[:, g * (G // 16) + q0 * 8:
                                           g * (G // 16) + q1 * 8],
                                        (q1 - q0) * 128, elem, queue_num=qn,
                                        single_packet=sp)
                            if not no_oh:
                                oh = op_.tile([128, nt * 128], tdt, tag="oh")
                                oh3 = oh[:].rearrange("p (t w) -> p t w", w=128)
                                nc.vector.tensor_tensor(
                                    out=oh3,
                                    in0=dw[:, g * nt:(g + 1) * nt].unsqueeze(2)
                                        .broadcast_to([128, nt, 128]),
                                    in1=io128b[:].unsqueeze(1)
                                        .broadcast_to([128, nt, 128]),
                                    op=OP.is_equal)
                            else:
                                oh3 = io128b[:].unsqueeze(1).broadcast_to(
                                    [128, nt, 128])
                            for cell in range(CPG):
                                j = g * CPG + cell
                                ps = psE.tile([128, elem], F32, tag="cell")
                                ps3 = ps[:].unsqueeze(1)
                                if not no_mm:
                                    for t in range(ntc):
                                        tt = cell * ntc + t
                                        nc.tensor.matmul(
                                            ps3, oh3[:, tt:tt + 1, :],
                                            tok3[:, tt:tt + 1, :],
                                            start=(t == 0), stop=(t == ntc - 1))
                                if not (no_acc or no_mm):
                                    nc.vector.tensor_tensor(
                                        out=acc3[:, j:j + 1, :],
                                        in0=acc3[:, j:j + 1, :],
                                        in1=ps3, op=OP.add)

            # ---------- Phase B: L1 ----------
            if do_l1 and upto in ('B', 'C', 'D', 'full'):
                edge_phase(y_tab, t1_sb, 1)

            # ---------- Phase C: z for own nodes ----------
            do_c = upto in ('C', 'D', 'full')
            cp_pool = tc.tile_pool(name="phaseC", bufs=1)
            ep = cp_pool.__enter__()
            zsb = cp.tile([128, NBLK * 32], F32)
            zs3 = zsb[:].rearrange("p (f e) -> p f e", e=32)
            if not do_c:
                nc.vector.memset(zsb[:], 0.0)
            if do_c:
                d2 = ep.tile([128, NBLK], F32)
                nc.vector.tensor_tensor(out=d2[:], in0=diso[:], in1=diso[:], op=OP.mult)
                nc.vector.tensor_tensor(out=d2[:], in0=d2[:], in1=xo[:], op=OP.mult)
                s = ep.tile([128, NBLK], F32)
                nc.vector.tensor_tensor(out=s[:], in0=t1_sb[:], in1=diso[:],
                                        op=OP.mult)
                nc.vector.tensor_tensor(out=s[:], in0=s[:], in1=d2[:], op=OP.add)

                # s -> single-partition row (node-order) via SBUF->SBUF DMA
                s_row = ep.tile([1, CORE_N], F32)
                nc.sync.dma_start(out=s_dram[:].rearrange("(f p) -> p f", p=128),
                                  in_=s[:])
                nc.sync.dma_start(out=s_row[:], in_=s_dram[:].unsqueeze(0))

                # h1T[k, n] = relu(w1[k]*s[n] + b1[k]) built 512 nodes at a time
                h1rT = ep.tile([64, CORE_N], BF16)
                with tc.tile_pool(name="psH", bufs=3, space="PSUM") as psH:
                    for j0 in range(0, CORE_N, 512):
                        n = min(512, CORE_N - j0)
                        h1_ps = psH.tile([64, 512], F32, tag="h1")
                        nc.tensor.matmul(h1_ps[:, 0:n], w1r[:],
                                         s_row[:, j0:j0 + n],
                                         start=True, stop=True)
                        nc.scalar.activation(h1rT[:, j0:j0 + n],
                                             h1_ps[:, 0:n], AF.Relu, bias=b1c[:],
                                             scale=1.0)

                # z[n,:] = diso[n] * (h1r @ W2)[n,:]
                z2 = ep.tile([128, NBLK * 32], BF16)
                z23 = z2[:].rearrange("p (f e) -> p f e", e=32)
                with tc.tile_pool(name="psC", bufs=3, space="PSUM") as psC:
                    for f in range(NBLK):
                        z_ps = psC.tile([128, 32], F32, tag="zps")
                        nc.tensor.matmul(z_ps[:], h1rT[:, 128 * f:128 * (f + 1)],
                                         W2b[:], start=True, stop=True)
                        nc.vector.tensor_scalar(out=zs3[:, f:f + 1, :],
                                                in0=z_ps[:].unsqueeze(1),
                                                scalar1=diso[:, f:f + 1], scalar2=None,
                                                op0=OP.mult)
                        nc.vector.tensor_copy(z23[:, f:f + 1, :], zs3[:, f:f + 1, :])
                # packed 4 nodes per 256B z_tab row; z_own[f*128+p, e] has
                # flat offset f*4096 + 32p + e = the packed-row address
                nc.sync.dma_start(
                    out=z_own[:, :].rearrange("(f p) e -> p f e", p=128),
                    in_=z2[:].rearrange("p (f e) -> p f e", e=32))
                nc.gpsimd.collective_compute(
                    "AllGather", OP.bypass, replica_groups=[list(range(8))],
                    ins=[z_own[:, :].opt()], outs=[z_tab[:, :].opt()])
            cp_pool.__exit__(None, None, None)

            # ---------- Phase D: L2 ----------
            if do_l2 and upto in ('D', 'full'):
                edge_phase(z_tab, t2_sb, 32)

            # ---------- Phase E: h2, pooling, MLP ----------
            ep_pool = tc.tile_pool(name="phaseE", bufs=1)
            ep = ep_pool.__enter__()
            t23 = t2_sb[:].rearrange("p (f e) -> p f e", e=32)
            hf = ep.tile([128, NBLK * 32], F32)
            hf3 = hf[:].rearrange("p (f e) -> p f e", e=32)
            nc.vector.tensor_tensor(out=hf3, in0=t23, in1=zs3, op=OP.add)
            nc.vector.tensor_tensor(out=hf3, in0=hf3,
                                    in1=diso[:].unsqueeze(2)
                                    .broadcast_to([128, NBLK, 32]), op=OP.mult)
            nc.vector.tensor_tensor(out=hf3, in0=hf3,
                                    in1=b2b[:].unsqueeze(1)
                                    .broadcast_to([128, NBLK, 32]), op=OP.add)
            nc.vector.tensor_scalar(out=hf[:], in0=hf[:], scalar1=0.0, scalar2=None,
                                    op0=OP.max)
            hfh = ep.tile([128, NBLK * 32], mybir.dt.float16)
            nc.vector.tensor_copy(hfh[:], hf[:])
            hfh3 = hfh[:].rearrange("p (f e) -> p f e", e=32)

            with tc.tile_pool(name="psP", bufs=1, space="PSUM") as psP, \
                 tc.tile_pool(name="psM", bufs=2, space="PSUM") as psM:
                pool_ps = psP.tile([32, 512], F32)
                for f in range(NBLK):
                    oh = wp.tile([128, 512], mybir.dt.float16, tag="poh")
                    nc.vector.tensor_scalar(out=oh[:], in0=io512[:],
                                            scalar1=bidc[:, f:f + 1], scalar2=None,
                                            op0=OP.is_equal)
                    nc.tensor.matmul(pool_ps[:], hfh3[:, f, :], oh[:],
                                     start=(f == 0), stop=(f == NBLK - 1))
                poolsb = ep.tile([32, 512], F32)
                nc.scalar.activation(poolsb[:], pool_ps[:], AF.Copy)
                nc.sync.dma_start(out=cc_in[:, :], in_=poolsb[:])
                nc.gpsimd.collective_compute(
                    "AllGather", OP.bypass, replica_groups=[list(range(8))],
                    ins=[cc_in[:, :].opt()], outs=[cc_out[:, :].opt()])

                pooled = ep.tile([32, G_ASM], F32)
                nc.vector.memset(pooled[:], 0.0)
                for c2 in range(8):
                    slab = wp.tile([32, 512], F32, tag="slab")
                    nc.sync.dma_start(out=slab[:],
                                      in_=cc_out[32 * c2:32 * (c2 + 1), :])
                    g0 = g_first[c2]
                    nc.vector.tensor_tensor(out=pooled[:, g0:g0 + 512],
                                            in0=pooled[:, g0:g0 + 512],
                                            in1=slab[:], op=OP.add)

                cnt = ep.tile([1, G_PAD], F32)
                nc.sync.dma_start(out=cnt[:], in_=counts[:].unsqueeze(0))
                nc.vector.tensor_scalar(out=cnt[:], in0=cnt[:], scalar1=1.0,
                                        scalar2=None, op0=OP.max)
                crec = ep.tile([1, G_PAD], F32)
                nc.vector.reciprocal(crec[:], cnt[:])
                crep = ep.tile([32, G_PAD], F32)
                for j in range(4):
                    cr_ps = psM.tile([32, 512], F32, tag="mm")
                    nc.tensor.matmul(cr_ps[:], ones1[:, 0:32],
                                     crec[:, 512 * j:512 * (j + 1)],
                                     start=True, stop=True)
                    nc.scalar.activation(crep[:, 512 * j:512 * (j + 1)], cr_ps[:],
                                         AF.Copy)
                pm = ep.tile([32, G_PAD], F32)
                nc.vector.tensor_tensor(out=pm[:], in0=pooled[:, 0:G_PAD],
                                        in1=crep[:], op=OP.mult)

                m1 = ep.tile([128, G_PAD], F32)
                for j in range(4):
                    m1_ps = psM.tile([128, 512], F32, tag="mm")
                    nc.tensor.matmul(m1_ps[:], Wp1sb[:],
                                     pm[:, 512 * j:512 * (j + 1)],
                                     start=True, stop=True)
                    nc.scalar.activation(m1[:, 512 * j:512 * (j + 1)], m1_ps[:],
                                         AF.Relu, bias=bp1c[:], scale=1.0)
                osb = ep.tile([3, G_PAD], F32)
                for j in range(4):
                    m2_ps = psM.tile([128, 512], F32, tag="mm")
                    nc.tensor.matmul(m2_ps[0:3, :], Wp2sb[:],
                                     m1[:, 512 * j:512 * (j + 1)],
                                     start=True, stop=True)
                    nc.vector.tensor_scalar(out=osb[:, 512 * j:512 * (j + 1)],
                                            in0=m2_ps[0:3, :], scalar1=bp2c[:],
                                            scalar2=None, op0=OP.add)
            nc.sync.dma_start(out=out[:, :].rearrange("g e -> e g"),
                              in_=osb[:, 0:N_GRAPHS])
            ep_pool.__exit__(None, None, None)
    nc.compile()
    return nc


# ---------------- host-side preprocessing ----------------

def prep_host(edge_index, batch):
    """Bucket edges into (class=src%4, block) cells; build idx/weight streams."""
    src = np.asarray(edge_index[0], np.int64)
    dst = np.asarray(edge_index[1], np.int64)
    batch = np.asarray(batch, np.int64)
    indeg = np.bincount(dst, minlength=NN).astype(np.int32)
    NCELL = N_CLASSES * NBLK

    core_of = dst // CORE_N
    per_core_raw = []
    ntc = 1
    for c in range(8):
        m = core_of == c
        s_c = src[m]
        dl = dst[m] - c * CORE_N
        klass = s_c % N_CLASSES
        block = dl >> 7
        off = dl & 127
        cell = klass * NBLK + block
        cnts = np.bincount(cell, minlength=NCELL)
        ntc = max(ntc, int(-(-cnts.max() // 128)))
        per_core_raw.append((s_c, off, cell, cnts))

    cap = ntc * 128
    g_first = []
    per_core = []
    for c in range(8):
        s_c, off, cell, cnts = per_core_raw[c]
        # sort by (cell, src) so each cell's gather sweeps the table in
        # ascending row order
        order = np.lexsort((s_c, cell))
        cs = cell[order]
        base = np.zeros(NCELL + 1, np.int64)
        np.cumsum(cnts, out=base[1:])
        rank = np.arange(len(cs)) - base[cs]
        pos = cs * cap + rank
        es_stream = np.zeros(NCELL * cap, np.int16)
        es_stream[pos] = (s_c[order] // N_CLASSES).astype(np.int16)
        dw_stream = np.full(NCELL * cap, -1.0, ml_dtypes.bfloat16)
        dw_stream[pos] = off[order].astype(ml_dtypes.bfloat16)

        esw = np.tile(es_stream.reshape(N_CLASSES, -1, 16).transpose(0, 2, 1),
                      (1, 8, 1)).copy()
        dww = dw_stream.reshape(N_CLASSES, -1, 128).transpose(0, 2, 1).copy()

        nb_real = min(CORE_N, N_NODES - c * CORE_N)
        bid_own = np.full(CORE_N, -1.0, np.float32)
        gf = int(batch[c * CORE_N])
        bid_own[:nb_real] = (batch[c * CORE_N:c * CORE_N + nb_real] - gf).astype(
            np.float32)
        assert bid_own.max() < 512
        g_first.append(gf)
        per_core.append(dict(esrc=esw, dstw=dww,
                             bid_p=np.ascontiguousarray(
                                 bid_own.reshape(NBLK, 128).T),
                             nb_real=nb_real))

    counts = np.bincount(batch, minlength=G_PAD).astype(np.float32)[:G_PAD]
    dis = ((indeg.astype(np.float64) + 1.0) ** -0.5).astype(np.float32)
    return per_core, dict(indeg=indeg, dis=dis, counts=counts, g_first=g_first,
                          ntc=ntc)


def make_inmaps(inputs, per_core, uniform):
    """Build per-core in_maps from full problem inputs + prep results."""
    FP = NN // 512 + 1
    x = np.asarray(inputs["x"], np.float32).reshape(-1)
    x_pad = np.zeros(FP * 512, np.float32)
    x_pad[:N_NODES] = x
    dis_pad = np.ones(FP * 512, np.float32)
    dis_pad[:NN] = uniform["dis"][:NN]

    def pack_g(v):
        # node n=512F+4P+m -> [P, 4F+m]
        return np.ascontiguousarray(
            v.reshape(FP, 128, 4).transpose(1, 0, 2).reshape(128, FP * 4)
            [:, :NCOLS + 2])

    common = dict(
        xg_p=pack_g(x_pad), disg_p=pack_g(dis_pad),
        counts=uniform["counts"],
        w1=np.asarray(inputs["W1"], np.float32).reshape(64),
        b1=np.asarray(inputs["b1"], np.float32),
        W2=np.asarray(inputs["W2"], np.float32),
        b2=np.asarray(inputs["b2"], np.float32),
        Wp1=np.asarray(inputs["Wp1"], np.float32),
        bp1=np.asarray(inputs["bp1"], np.float32),
        Wp2=np.asarray(inputs["Wp2"], np.float32),
        bp2=np.asarray(inputs["bp2"], np.float32),
    )
    in_maps = []
    for c in range(8):
        pc = per_core[c]
        xo = np.zeros(CORE_N, np.float32)
        nb = pc["nb_real"]
        xo[:nb] = x_pad[c * CORE_N:c * CORE_N + nb]
        dso = np.ones(CORE_N, np.float32)
        dso[:nb] = dis_pad[c * CORE_N:c * CORE_N + nb]
        in_maps.append(dict(
            common,
            xo_p=np.ascontiguousarray(xo.reshape(NBLK, 128).T),
            diso_p=np.ascontiguousarray(dso.reshape(NBLK, 128).T),
            bid_p=pc["bid_p"], esrc=pc["esrc"], dstw=pc["dstw"]))
    return in_maps


# ---------------- harness entry point ----------------

_CACHE = {}


def kernel(**inputs):
    """Full-input GCN forward on 8 trn2 NeuronCores; returns [2000, 3] f32."""
    from concourse.bass_utils import run_bass_kernel_spmd
    inputs = {k: np.asarray(v) for k, v in inputs.items()}
    per_core, uniform = prep_host(inputs["edge_index"], inputs["batch"])
    key = (tuple(uniform["g_first"]), uniform["ntc"])
    if key not in _CACHE:
        _CACHE[key] = build_nc(uniform["g_first"], uniform["ntc"])
    nc = _CACHE[key]
    in_maps = make_inmaps(inputs, per_core, uniform)
    res = run_bass_kernel_spmd(nc, in_maps, core_ids=list(range(8)))
    return np.ascontiguousarray(res.results[0]["out"].astype(np.float32))
